# revision 1
# baseline (speedup 1.0000x reference)
"""Trainium2 Bass kernel for multi-head attention with RoPE (B=2, S=2048,
D=2048, H=16), distributed over 8 NeuronCores with head tensor-parallelism
and an AllToAll to switch to token-parallelism for the output projection.

kernel(**inputs) takes the full unsharded inputs (as produced by the
reference setup_inputs) and returns the full [2, 2048, 2048] f32 output.
"""
import numpy as np
import ml_dtypes
from concourse import bass, bacc, tile, mybir
from concourse.bass_utils import run_bass_kernel_spmd

bf16 = ml_dtypes.bfloat16
BF16 = mybir.dt.bfloat16
F32 = mybir.dt.float32
AF = mybir.ActivationFunctionType
OP = mybir.AluOpType

B, S, D, H = 2, 2048, 2048, 16
HD = 128                 # head dim
NCORES = 8
HL = H // NCORES         # heads per core = 2
EL = HL * HD             # local projection width = 256
T = B * S                # 4096 flattened tokens
TCH = 512                # token chunk for QKV phase
NTCH = T // TCH          # 8
NKT = S // 128           # 16 key tiles per batch
NQC = S // 512           # 4 query chunks per batch
NDT = D // 128           # 16 contraction tiles
TL = T // NCORES         # 512 tokens per core after AllToAll
SCALE = float(1.0 / np.sqrt(128.0))

_CACHE = {}
_DEBUG = False


def _build():
    nc = bacc.Bacc("TRN2", target_bir_lowering=False, num_devices=NCORES)
    dbg = {}
    if _DEBUG:
        dbg["qT0"] = nc.dram_tensor("dbg_qT0", [128, T], BF16, kind="ExternalOutput")
        dbg["kT0"] = nc.dram_tensor("dbg_kT0", [128, T], BF16, kind="ExternalOutput")
        dbg["vn0"] = nc.dram_tensor("dbg_vn0", [128, T], BF16, kind="ExternalOutput")
        dbg["xt0"] = nc.dram_tensor("dbg_xt0", [128, TCH], BF16, kind="ExternalOutput")
        dbg["E0"] = nc.dram_tensor("dbg_E0", [128, S], BF16, kind="ExternalOutput")
        dbg["on0"] = nc.dram_tensor("dbg_on0", [128, 512], BF16, kind="ExternalOutput")
        dbg["rec0"] = nc.dram_tensor("dbg_rec0", [128, 512], F32, kind="ExternalOutput")
        dbg["ot0"] = nc.dram_tensor("dbg_ot0", [128, TL], BF16, kind="ExternalOutput")
        dbg["a2ain"] = nc.dram_tensor("dbg_a2ain", [NCORES, EL, TL], BF16, kind="ExternalOutput")
        dbg["a2aout"] = nc.dram_tensor("dbg_a2aout", [NCORES, EL, TL], BF16, kind="ExternalOutput")
        dbg["y0"] = nc.dram_tensor("dbg_y0", [128, 512], F32, kind="ExternalOutput")

    x = nc.dram_tensor("x", [T, D], F32, kind="ExternalInput")
    wq_t = nc.dram_tensor("wq_t", [D, EL], BF16, kind="ExternalInput")
    wk_t = nc.dram_tensor("wk_t", [D, EL], BF16, kind="ExternalInput")
    wv_t = nc.dram_tensor("wv_t", [D, EL], BF16, kind="ExternalInput")
    wo_t = nc.dram_tensor("wo_t", [D, D], BF16, kind="ExternalInput")
    cos_t = nc.dram_tensor("cos_t", [HD, S], BF16, kind="ExternalInput")
    sin_m = nc.dram_tensor("sin_m", [HD, S], BF16, kind="ExternalInput")
    mask_t = nc.dram_tensor("mask_t", [128, B * NKT], F32, kind="ExternalInput")
    out = nc.dram_tensor("out", [TL, D], F32, kind="ExternalOutput")

    ident_dram = nc.inline_tensor(np.eye(128, dtype=bf16), name="ident")
    ones_dram = nc.inline_tensor(np.ones((128, 128), dtype=bf16), name="ones")

    with tile.TileContext(nc) as tc:
        with (
            # ---------- persistent pools (whole kernel) ----------
            tc.tile_pool(name="dram", bufs=1, space="DRAM") as dram,
            tc.tile_pool(name="consts", bufs=1) as consts,
            tc.tile_pool(name="qkv_keep", bufs=1) as keep,
        ):
            xbf = [dram.tile([1024, D], BF16, tag=f"xbf{c}", name=f"xbf{c}")
                   for c in range(T // 1024)]
            a2a_in = [dram.tile([NCORES, HD, TL], BF16, tag=f"a2a_in{h}", name=f"a2a_in{h}")
                      for h in range(HL)]
            a2a_out = [dram.tile([NCORES, HD, TL], BF16, tag=f"a2a_out{h}", name=f"a2a_out{h}")
                       for h in range(HL)]

            ident_sb = consts.tile([128, 128], BF16, tag="ident", name="ident_sb")
            nc.sync.dma_start(ident_sb[:], ident_dram[:])
            ones_sb = consts.tile([128, 128], BF16, tag="ones", name="ones_sb")
            nc.sync.dma_start(ones_sb[:], ones_dram[:])
            mask_sb = consts.tile([128, B * NKT], F32, tag="mask", name="mask_sb")
            nc.sync.dma_start(mask_sb[:], mask_t[:])

            # persistent per-head tensors ([hd, t] layouts; v natural packed
            # as 32 [t=128, hd=128] tiles along free dim)
            qT = [keep.tile([128, T], BF16, tag=f"qT{h}", name=f"qT{h}") for h in range(HL)]
            kT = [keep.tile([128, T], BF16, tag=f"kT{h}", name=f"kT{h}") for h in range(HL)]
            vnat = [keep.tile([128, T], BF16, tag=f"vn{h}", name=f"vn{h}") for h in range(HL)]

            # ---------- phase A+B: x prep + QKV projections + RoPE ----------
            with (
                tc.tile_pool(name="wsb", bufs=1) as wpool,
                tc.tile_pool(name="tables", bufs=1) as tbl,
                tc.tile_pool(name="xstage", bufs=4) as xstage,
                tc.tile_pool(name="xbstage", bufs=3) as xbstage,
                tc.tile_pool(name="xt", bufs=20) as xtpool,
                tc.tile_pool(name="rope", bufs=2) as rope,
                tc.tile_pool(name="vT", bufs=1) as vTpool,
                tc.tile_pool(name="qkvps", bufs=6, space="PSUM") as qkvps,
                tc.tile_pool(name="tps", bufs=2, space="PSUM") as tps,
            ):
                vT = [vTpool.tile([128, T], BF16, tag=f"vT{h}", name=f"vT{h}") for h in range(HL)]
                wsb = {}
                for nm, wt in (("q", wq_t), ("k", wk_t), ("v", wv_t)):
                    for d in range(NDT):
                        wtile = wpool.tile([128, EL], BF16, tag=f"w{nm}{d}", name=f"w{nm}{d}")
                        nc.gpsimd.dma_start(wtile[:], wt[d * 128:(d + 1) * 128, :])
                        wsb[(nm, d)] = wtile
                cos_sb = tbl.tile([128, S], BF16, tag="cos", name="cos_sb")
                nc.gpsimd.dma_start(cos_sb[:], cos_t[:])
                sin_sb = tbl.tile([128, S], BF16, tag="sin", name="sin_sb")
                nc.gpsimd.dma_start(sin_sb[:], sin_m[:])


                TG2 = 1024
                for tg2 in range(T // TG2):   # 1024-token groups
                    g0 = tg2 * TG2
                    # stage x: load f32, cast to bf16, store to DRAM scratch
                    for tt in range(TG2 // 128):
                        r0 = g0 + tt * 128
                        xf = xstage.tile([128, D], F32, tag="xf", name="xf")
                        nc.gpsimd.dma_start(xf[:], x[r0:r0 + 128, :])
                        xb = xbstage.tile([128, D], BF16, tag="xb", name="xb")
                        nc.scalar.add(xb[:], xf[:], 0.0)
                        nc.scalar.dma_start(
                            xbf[tg2][tt * 128:(tt + 1) * 128, :], xb[:])
                    # transpose group into SBUF: xt[d] = xbf[:, d*128:+128].T
                    xts = []
                    for d in range(NDT):
                        xtile = xtpool.tile([128, TG2], BF16, tag="xt", name="xt")
                        nc.sync.dma_start(
                            xtile[:], xbf[tg2][:, d * 128:(d + 1) * 128],
                            transpose=True)
                        xts.append(xtile)
                    # QKV matmuls per 512-token half (d-outer: frees xt fast)
                    for half in range(2):
                        t0 = g0 + half * TCH
                        hs = half * TCH
                        scol = ((t0 // TCH) % NQC) * TCH
                        pss = {}
                        for nm in ("q", "k", "v"):
                            for eh in range(HL):
                                pss[(nm, eh)] = qkvps.tile(
                                    [128, TCH], F32, tag="qkvps", name="qkvps")
                        for d in range(NDT):
                            for nm in ("q", "k", "v"):
                                for eh in range(HL):
                                    nc.tensor.matmul(
                                        pss[(nm, eh)][:],
                                        wsb[(nm, d)][:, eh * 128:(eh + 1) * 128],
                                        xts[d][:, hs:hs + TCH],
                                        start=(d == 0), stop=(d == NDT - 1))
                        for nm in ("q", "k", "v"):
                            for eh in range(HL):
                                ps = pss[(nm, eh)]
                                if nm == "v":
                                    nc.vector.tensor_copy(
                                        vT[eh][:, t0:t0 + TCH], ps[:])
                                else:
                                    dst = qT[eh] if nm == "q" else kT[eh]
                                    tmp = rope.tile([128, TCH], F32, tag="ropetmp", name="ropetmp")
                                    nc.vector.tensor_tensor(
                                        tmp[:], ps[:], cos_sb[:, scol:scol + TCH],
                                        OP.mult)
                                    u = rope.tile([128, TCH], F32, tag="ropeu", name="ropeu")
                                    nc.vector.tensor_tensor(
                                        u[0:64, :], ps[64:128, :],
                                        sin_sb[0:64, scol:scol + TCH], OP.mult)
                                    nc.vector.tensor_tensor(
                                        u[64:128, :], ps[0:64, :],
                                        sin_sb[64:128, scol:scol + TCH], OP.mult)
                                    nc.vector.tensor_tensor(
                                        dst[:, t0:t0 + TCH], tmp[:], u[:], OP.add)
                # batched v transposes to natural [t, hd] layout
                for eh in range(HL):
                    for ktg in range(T // 128):
                        c0 = ktg * 128
                        tp = tps.tile([128, 128], BF16, tag="tps", name="tpsum")
                        nc.tensor.transpose(
                            tp[:], vT[eh][:, c0:c0 + 128], ident_sb[:])
                        nc.vector.tensor_copy(vnat[eh][:, c0:c0 + 128], tp[:])
                if _DEBUG:
                    nc.sync.dma_start(dbg["qT0"][:], qT[0][:])
                    nc.sync.dma_start(dbg["kT0"][:], kT[0][:])
                    nc.sync.dma_start(dbg["vn0"][:], vnat[0][:])

            with tc.tile_pool(name="wo", bufs=1) as wopool:
                wo_sb = []
                for d in range(NDT):
                    wtile = wopool.tile([128, D], BF16, tag=f"wo{d}", name=f"wo{d}")
                    nc.gpsimd.dma_start(wtile[:], wo_t[d * 128:(d + 1) * 128, :])
                    wo_sb.append(wtile)
                # ---------- phase C: SDPA per (batch, head) ----------
                otpool = tc.alloc_tile_pool(name="ot", bufs=1)
                ot_sb = []
                with (
                    tc.tile_pool(name="E", bufs=NKT + 2) as epool,
                    tc.tile_pool(name="onorm", bufs=4) as onpool,
                    tc.tile_pool(name="rec", bufs=4) as recpool,
                    tc.tile_pool(name="sps", bufs=2, space="PSUM") as spool,
                    tc.tile_pool(name="ops", bufs=2, space="PSUM") as opool,
                    tc.tile_pool(name="dps", bufs=2, space="PSUM") as dpool,
                ):
                    for h in range(HL):
                        for b in range(B):
                            q0 = b * S
                            for qp in range(2):     # qt half: 1024 queries
                                qb = q0 + qp * 1024
                                E = []
                                ops, dps = [], []
                                for qc2 in range(2):
                                    ops.append(opool.tile([128, 512], F32, tag="ops", name="opsum"))
                                    dps.append(dpool.tile([128, 512], F32, tag="dps", name="dpsum"))
                                def attn_step(kt):
                                    e_t = E[kt]
                                    vsl = vnat[h][:, (b * NKT + kt) * 128:(b * NKT + kt + 1) * 128]
                                    for qc2 in range(2):
                                        erhs = e_t[:, qc2 * 512:(qc2 + 1) * 512]
                                        nc.tensor.matmul(
                                            ops[qc2][:], vsl, erhs,
                                            start=(kt == 0), stop=(kt == NKT - 1))
                                        nc.tensor.matmul(
                                            dps[qc2][:], ones_sb[:], erhs,
                                            start=(kt == 0), stop=(kt == NKT - 1))
                                for kt in range(NKT):
                                    sp = spool.tile([128, 1024], F32, tag="sps", name="spsum")
                                    for qh in range(2):
                                        nc.tensor.matmul(
                                            sp[:, qh * 512:(qh + 1) * 512],
                                            kT[h][:, q0 + kt * 128:q0 + (kt + 1) * 128],
                                            qT[h][:, qb + qh * 512:qb + (qh + 1) * 512],
                                            start=True, stop=True)
                                    e_t = epool.tile([128, 1024], BF16, tag="E", name="etile")
                                    mcol = b * NKT + kt
                                    nc.scalar.activation(
                                        e_t[:], sp[:], AF.Exp,
                                        bias=mask_sb[:, mcol:mcol + 1],
                                        scale=SCALE)
                                    E.append(e_t)
                                    if kt > 0:
                                        attn_step(kt - 1)
                                attn_step(NKT - 1)
                                for qc2 in range(2):
                                    qc = qp * 2 + qc2
                                    rec = recpool.tile([128, 512], F32, tag="rec", name="rec")
                                    nc.vector.reciprocal(rec[:], dps[qc2][:])
                                    on = onpool.tile([128, 512], BF16, tag="on", name="onorm")
                                    nc.vector.tensor_tensor(on[:], ops[qc2][:], rec[:], OP.mult)
                                    j = b * NQC + qc
                                    nc.gpsimd.dma_start(a2a_in[h][j, :, :], on[:])
                        # fire this head's AllToAll (head 0's overlaps head 1 SDPA)
                        nc.gpsimd.collective_compute(
                            "AllToAll", OP.bypass,
                            replica_groups=[list(range(NCORES))],
                            ins=[a2a_in[h].opt()],
                            outs=[a2a_out[h].opt()],
                        )
                    # OT loads: h0 tiles start right after the first AllToAll
                    for hh in range(HL):
                        for d in range(NDT):
                            if d % 2 != hh:
                                continue
                            otile = otpool.tile([128, TL], BF16, tag=f"ot{d}", name=f"ot{d}")
                            nc.sync.dma_start(
                                otile[:], a2a_out[d % 2][d // 2, :, :])
                            ot_sb.append((d, otile))
                    ot_sb = [t for _, t in sorted(ot_sb)]

                # ---------- phase D: output projection for my 512 tokens ----------
                with (
                    tc.tile_pool(name="ysb", bufs=8) as ypool,
                    tc.tile_pool(name="yps", bufs=8, space="PSUM") as ypsp,
                ):
                    for tt in range(TL // 128):
                        yps = [ypsp.tile([128, 512], F32, tag="yps", name="ypsum")
                               for _ in range(4)]
                        for d in range(NDT):
                            for eo in range(4):
                                nc.tensor.matmul(
                                    yps[eo][:],
                                    ot_sb[d][:, tt * 128:(tt + 1) * 128],
                                    wo_sb[d][:, eo * 512:(eo + 1) * 512],
                                    start=(d == 0), stop=(d == NDT - 1))
                        for eo in range(4):
                            ysb = ypool.tile([128, 512], F32, tag="ysb", name="ysb")
                            nc.vector.tensor_copy(ysb[:], yps[eo][:])
                            nc.gpsimd.dma_start(
                                out[tt * 128:(tt + 1) * 128,
                                    eo * 512:(eo + 1) * 512], ysb[:])
                otpool.release()

    nc.compile()
    return nc


def _prep_in_maps(x, cos, sin, attn_mask, wq, wk, wv, wo):
    xf = np.ascontiguousarray(x.reshape(T, D).astype(np.float32, copy=False))
    cosT = np.ascontiguousarray(np.asarray(cos[0], np.float32).T)   # [HD, S]
    sinT = np.asarray(sin[0], np.float32).T
    sin_m = np.ascontiguousarray(
        np.concatenate([-sinT[:64], sinT[64:]], axis=0))            # [HD, S]
    mask_t = np.ascontiguousarray(
        np.asarray(attn_mask, np.float32).reshape(B * NKT, 128).T)  # [128, 32]
    wo_t = np.ascontiguousarray(np.asarray(wo, np.float32).T.astype(bf16))
    in_maps = []
    for i in range(NCORES):
        sl = slice(i * EL, (i + 1) * EL)
        in_maps.append({
            "x": xf,
            "wq_t": np.ascontiguousarray(np.asarray(wq, np.float32)[sl].T.astype(bf16)),
            "wk_t": np.ascontiguousarray(np.asarray(wk, np.float32)[sl].T.astype(bf16)),
            "wv_t": np.ascontiguousarray(np.asarray(wv, np.float32)[sl].T.astype(bf16)),
            "wo_t": wo_t,
            "cos_t": cosT.astype(bf16),
            "sin_m": sin_m.astype(bf16),
            "mask_t": mask_t,
        })
    return in_maps


def kernel(x, cos, sin, attn_mask, wq, wk, wv, wo, _trace=False):
    if "nc" not in _CACHE:
        _CACHE["nc"] = _build()
    nc = _CACHE["nc"]
    in_maps = _prep_in_maps(x, cos, sin, attn_mask, wq, wk, wv, wo)
    res = run_bass_kernel_spmd(nc, in_maps, core_ids=list(range(NCORES)),
                               trace=_trace)
    _CACHE["last_result"] = res
    y = np.concatenate([np.asarray(res.results[i]["out"], np.float32)
                        for i in range(NCORES)], axis=0)
    return y.reshape(B, S, D)



# revision 25
# speedup vs baseline: 1.4062x; 1.4062x over previous
"""Trainium2 Bass kernel for multi-head attention with RoPE (B=2, S=2048,
D=2048, H=16), distributed over 8 NeuronCores as 2-way batch data parallel
x 4-way head tensor parallel.  Each core computes QKV+RoPE+SDPA for its 4
heads of its batch, then an AllToAll (per 1024-query chunk, within each
4-core batch group) switches to token parallelism for the output
projection.  Each core returns the [512, 2048] slice for the tokens it
owns; the host reassembles the full [2, 2048, 2048] output.

kernel(**inputs) takes the full unsharded inputs (as produced by the
reference setup_inputs) and returns the full output.
"""
import numpy as np
import ml_dtypes
from concourse import bass, bacc, tile, mybir
from concourse.bass_utils import run_bass_kernel_spmd

bf16 = ml_dtypes.bfloat16
BF16 = mybir.dt.bfloat16
F32 = mybir.dt.float32
AF = mybir.ActivationFunctionType
OP = mybir.AluOpType

B, S, D, H = 2, 2048, 2048, 16
HD = 128                   # head dim
NCORES = 8
GSZ = 4                    # tensor-parallel group size (cores per batch)
HL = H // GSZ              # heads per core = 4
EL = HL * HD               # local projection width = 512
TL = S                     # tokens per core in phase B/C (its whole batch)
NDT = D // 128             # 16 contraction tiles
NKT = S // 128             # 16 key tiles
NQC = 4                    # x-prep chunks of 512 tokens (= QKV matmul quarters)
NTT = TL // 128            # 16 token tiles
TOK = S // NCORES * B      # 512 tokens owned per core after AllToAll
SCALE = float(1.0 / np.sqrt(128.0))

_CACHE = {}
_DEBUG = False


def _build():
    nc = bacc.Bacc("TRN2", target_bir_lowering=False, num_devices=NCORES)
    dbg = {}
    if _DEBUG:
        dbg["qT0"] = nc.dram_tensor("dbg_qT0", [128, TL], BF16, kind="ExternalOutput")
        dbg["kT0"] = nc.dram_tensor("dbg_kT0", [128, TL], BF16, kind="ExternalOutput")
        dbg["vn0"] = nc.dram_tensor("dbg_vn0", [128, TL], BF16, kind="ExternalOutput")
        dbg["E0"] = nc.dram_tensor("dbg_E0", [128, 1024], BF16, kind="ExternalOutput")
        dbg["on0"] = nc.dram_tensor("dbg_on0", [128, 512], BF16, kind="ExternalOutput")

    x = nc.dram_tensor("x", [TL, D], F32, kind="ExternalInput")
    wq_t = nc.dram_tensor("wq_t", [128, NDT * EL], BF16, kind="ExternalInput")
    wk_t = nc.dram_tensor("wk_t", [128, NDT * EL], BF16, kind="ExternalInput")
    wv_t = nc.dram_tensor("wv_t", [128, NDT * EL], BF16, kind="ExternalInput")
    wo_t = nc.dram_tensor("wo_t", [128, NDT * D], BF16, kind="ExternalInput")
    cos_t = nc.dram_tensor("cos_t", [HD, S], BF16, kind="ExternalInput")
    sin_m = nc.dram_tensor("sin_m", [HD, S], BF16, kind="ExternalInput")
    mask_t = nc.dram_tensor("mask_t", [128, NKT], F32, kind="ExternalInput")
    out = nc.dram_tensor("out", [TOK, D], BF16, kind="ExternalOutput")

    ones_dram = nc.inline_tensor(np.ones((128, 128), dtype=bf16), name="ones")

    with tile.TileContext(nc) as tc:
        with (
            tc.tile_pool(name="dram", bufs=1, space="DRAM") as dram,
            tc.tile_pool(name="consts", bufs=1) as consts,
            tc.tile_pool(name="keep", bufs=1) as keep,
        ):
            xbf = dram.tile([TL, D], BF16, tag="xbf", name="xbf")
            a2a_in = dram.tile([NCORES, EL, 256], BF16, tag="a2a_in",
                               name="a2a_in")
            a2a_out = dram.tile([NCORES, EL, 256], BF16, tag="a2a_out",
                                name="a2a_out")

            ones_sb = consts.tile([128, 128], BF16, tag="ones", name="ones_sb")
            nc.gpsimd.dma_start(ones_sb[:], ones_dram[:])
            mask_sb = consts.tile([128, NKT], F32, tag="mask", name="mask_sb")
            nc.gpsimd.dma_start(mask_sb[:], mask_t[:])
            cos_sb = consts.tile([128, S], BF16, tag="cos", name="cos_sb")
            nc.gpsimd.dma_start(cos_sb[:], cos_t[:])
            sin_sb = consts.tile([128, S], BF16, tag="sin", name="sin_sb")
            nc.gpsimd.dma_start(sin_sb[:], sin_m[:])

            # persistent per-head tensors: q/k in [hd, t]; v natural as 16
            # [t=128, e=512] tiles
            qT = [keep.tile([128, TL], BF16, tag=f"qT{h}", name=f"qT{h}")
                  for h in range(HL)]
            kT = [keep.tile([128, TL], BF16, tag=f"kT{h}", name=f"kT{h}")
                  for h in range(HL)]
            vnat = [keep.tile([128, EL], BF16, tag=f"vn{t}", name=f"vn{t}")
                    for t in range(NTT)]

            # ---------- phase A+B: x prep + QKV projections + RoPE ----------
            with (
                tc.tile_pool(name="wsb", bufs=1) as wpool,
                tc.tile_pool(name="xt", bufs=2) as xtpool,
                tc.tile_pool(name="xstage", bufs=2) as xstage,
                tc.tile_pool(name="xbstage", bufs=2) as xbstage,
                tc.tile_pool(name="rope", bufs=2) as rope,
                tc.tile_pool(name="vps", bufs=2, space="PSUM") as vps,
                tc.tile_pool(name="qkps", bufs=4, space="PSUM") as qkps,
            ):
                # weights host-preswizzled to [128, d*EL+e]: one DMA each
                wsb = {}

                def load_w(nm, wt):
                    wtile = wpool.tile([128, NDT * EL], BF16, tag=f"w{nm}",
                                       name=f"w{nm}")
                    nc.gpsimd.dma_start(wtile[:], wt[:])
                    for d in range(NDT):
                        wsb[(nm, d)] = wtile[:, d * EL:(d + 1) * EL]

                for q in range(NQC):
                    r0 = q * 512
                    # ---- prep chunk q: load f32, cast (ScalarE), store,
                    # DMA-transpose (sync ring only) into per-(d,q) tiles
                    halves = []
                    for half in range(2):
                        rr = r0 + half * 256
                        xf = xstage.tile([128, 2 * D], F32, tag="xf", name="xf")
                        nc.gpsimd.dma_start(
                            xf[:].rearrange("p (s d) -> p s d", s=2),
                            x[rr:rr + 256, :].rearrange("(s p) d -> p s d", p=128))
                        halves.append((rr, xf))
                    if q == 0:
                        load_w("v", wv_t)
                        load_w("k", wk_t)
                        load_w("q", wq_t)
                    for rr, xfh in halves:
                        xb = xbstage.tile([128, 2 * D], BF16, tag="xb", name="xb")
                        nc.scalar.add(xb[:], xfh[:], 0.0)
                        nc.scalar.dma_start(
                            xbf[rr:rr + 256, :].rearrange("(s p) d -> p s d",
                                                          p=128),
                            xb[:].rearrange("p (s d) -> p s d", s=2))
                    xtq = []
                    for d in range(NDT):
                        # NOTE: transpose DMAs must stay on the sync ring —
                        # ACT-ring transposes race with TensorE consumers.
                        xtile = xtpool.tile([128, 512], BF16, tag=f"xt{d}",
                                            name=f"xt{d}")
                        nc.sync.dma_start(
                            xtile[:], xbf[r0:r0 + 512, d * 128:(d + 1) * 128],
                            transpose=True)
                        xtq.append(xtile)

                    # ---- compute chunk q: V natural tiles, then Q/K + RoPE
                    for tt in range(4 * q, 4 * q + 4):
                        vp = vps.tile([128, EL], F32, tag="vps", name="vpsum")
                        for d in range(NDT):
                            nc.tensor.matmul(
                                vp[:],
                                xtq[d][:, (tt % 4) * 128:(tt % 4 + 1) * 128],
                                wsb[("v", d)][:],
                                start=(d == 0), stop=(d == NDT - 1))
                        nc.vector.tensor_copy(vnat[tt][:], vp[:])
                    c0 = q * 512
                    for eh in range(HL):
                        for nm in ("k", "q"):
                            ps = qkps.tile([128, 512], F32, tag="qkps",
                                           name="qkps")
                            for d in range(NDT):
                                nc.tensor.matmul(
                                    ps[:],
                                    wsb[(nm, d)][:, eh * 128:(eh + 1) * 128],
                                    xtq[d][:],
                                    start=(d == 0), stop=(d == NDT - 1))
                            dst = qT[eh] if nm == "q" else kT[eh]
                            tmp = rope.tile([128, 512], F32, tag="ropetmp",
                                            name="ropetmp")
                            nc.vector.tensor_tensor(
                                tmp[:], ps[:], cos_sb[:, c0:c0 + 512], OP.mult)
                            u = rope.tile([128, 512], F32, tag="ropeu",
                                          name="ropeu")
                            nc.vector.tensor_tensor(
                                u[0:64, :], ps[64:128, :],
                                sin_sb[0:64, c0:c0 + 512], OP.mult)
                            nc.vector.tensor_tensor(
                                u[64:128, :], ps[0:64, :],
                                sin_sb[64:128, c0:c0 + 512], OP.mult)
                            nc.vector.tensor_tensor(
                                dst[:, c0:c0 + 512], tmp[:], u[:], OP.add)
            if _DEBUG:
                nc.sync.dma_start(dbg["qT0"][:], qT[0][:])
                nc.sync.dma_start(dbg["kT0"][:], kT[0][:])
                for t in range(NTT):
                    nc.sync.dma_start(dbg["vn0"][:, t * 128:(t + 1) * 128],
                                      vnat[t][:, 0:128])

            # ---------- phase C: SDPA + chunked AllToAll; phase D: out proj ----
            with tc.tile_pool(name="wo", bufs=1) as wopool:
                wo_big = wopool.tile([128, NDT * D], BF16, tag="wo", name="wo_big")
                nc.gpsimd.dma_start(wo_big[:], wo_t[:])
                wo_sb = [wo_big[:, d * D:(d + 1) * D] for d in range(NDT)]

                with (
                    tc.tile_pool(name="E", bufs=NKT + 2) as epool,
                    tc.tile_pool(name="eacc", bufs=2) as eaccpool,
                    tc.tile_pool(name="onorm", bufs=4) as onpool,
                    tc.tile_pool(name="rec", bufs=4) as recpool,
                    tc.tile_pool(name="ot", bufs=1) as otpool,
                    tc.tile_pool(name="ysb", bufs=2) as ypool,
                    tc.tile_pool(name="sps", bufs=2, space="PSUM") as spool,
                    tc.tile_pool(name="ops", bufs=2, space="PSUM") as opool,
                    tc.tile_pool(name="dps", bufs=1, space="PSUM") as dpool,
                    tc.tile_pool(name="yps", bufs=1, space="PSUM") as ypsp,
                ):
                    ot_sb = {}
                    for qp in range(2):
                        qb = qp * 1024
                        for h in range(HL):
                            E = []
                            ops = [opool.tile([128, 512], F32, tag="ops",
                                              name="opsum") for _ in range(2)]

                            def attn_step(kt):
                                e_t = E[kt]
                                vsl = vnat[kt][:, h * 128:(h + 1) * 128]
                                for qc2 in range(2):
                                    erhs = e_t[:, qc2 * 512:(qc2 + 1) * 512]
                                    nc.tensor.matmul(
                                        ops[qc2][:], vsl, erhs,
                                        start=(kt == 0), stop=(kt == NKT - 1))
                            for kt in range(NKT):
                                sp = spool.tile([128, 1024], F32, tag="sps",
                                                name="spsum")
                                for qh in range(2):
                                    nc.tensor.matmul(
                                        sp[:, qh * 512:(qh + 1) * 512],
                                        kT[h][:, kt * 128:(kt + 1) * 128],
                                        qT[h][:, qb + qh * 512:qb + (qh + 1) * 512],
                                        start=True, stop=True)
                                e_t = epool.tile([128, 1024], BF16, tag="E",
                                                 name="etile")
                                nc.scalar.activation(
                                    e_t[:], sp[:], AF.Exp,
                                    bias=mask_sb[:, kt:kt + 1], scale=SCALE)
                                E.append(e_t)
                                if kt > 0:
                                    attn_step(kt - 1)
                            attn_step(NKT - 1)
                            # denominators: accumulate E tiles on DVE (bf16),
                            # then one ones-matmul per query half
                            eacc = eaccpool.tile([128, 1024], BF16, tag="eacc",
                                                 name="eacc")
                            nc.vector.tensor_tensor(
                                eacc[:], E[0][:], E[1][:], OP.add)
                            for kt in range(2, NKT):
                                nc.vector.tensor_tensor(
                                    eacc[:], eacc[:], E[kt][:], OP.add)
                            for qc2 in range(2):
                                dps = dpool.tile([128, 512], F32, tag="dps",
                                                 name="dpsum")
                                nc.tensor.matmul(
                                    dps[:], ones_sb[:],
                                    eacc[:, qc2 * 512:(qc2 + 1) * 512],
                                    start=True, stop=True)
                                rec = recpool.tile([128, 512], F32, tag="rec",
                                                   name="rec")
                                nc.vector.reciprocal_approx_fast(
                                    out=rec[:], in_=dps[:])
                                on = onpool.tile([128, 512], BF16, tag="on",
                                                 name="onorm")
                                nc.vector.tensor_tensor(
                                    on[:], ops[qc2][:], rec[:], OP.mult)
                                nc.gpsimd.dma_start(
                                    a2a_in[qc2 * 4:(qc2 + 1) * 4,
                                           h * 128:(h + 1) * 128,
                                           qp * 128:(qp + 1) * 128
                                           ].rearrange("k e t -> e k t"),
                                    on[:].rearrange("p (k t) -> p k t", k=4))
                            if _DEBUG and qp == 0 and h == 0:
                                nc.sync.dma_start(dbg["E0"][:], E[0][:])
                                nc.sync.dma_start(dbg["on0"][:], on[:])
                        nc.gpsimd.collective_compute(
                            "AllToAll", OP.bypass,
                            replica_groups=[list(range(NCORES))],
                            ins=[a2a_in[qp].opt()],
                            outs=[a2a_out[qp].opt()],
                        )

                    # ---------- phase D ----------
                    with (
                        tc.tile_pool(name="ot", bufs=1) as otpool,
                        tc.tile_pool(name="ysb", bufs=2) as ypool,
                        tc.tile_pool(name="yps", bufs=4, space="PSUM") as ypsp,
                    ):
                        for qp in range(2):
                            for p in range(2):      # batch-part of my tokens
                                ot_sb = []
                                for dd in range(NDT):
                                    otile = otpool.tile([128, 128], BF16,
                                                        tag=f"ot{dd}",
                                                        name=f"ot{dd}")
                                    nc.gpsimd.dma_start(
                                        otile[:],
                                        a2a_out[qp][GSZ * p + dd // HL,
                                                    (dd % HL) * 128:
                                                    (dd % HL + 1) * 128, :])
                                    ot_sb.append(otile)
                                for ep in range(2):  # eo pairs: 2 psum banks
                                    yps = [ypsp.tile([128, 512], F32, tag="yps",
                                                     name="ypsum")
                                           for _ in range(2)]
                                    for d in range(NDT):
                                        for k in range(2):
                                            eo = ep * 2 + k
                                            nc.tensor.matmul(
                                                yps[k][:], ot_sb[d][:],
                                                wo_sb[d][:, eo * 512:(eo + 1) * 512],
                                                start=(d == 0), stop=(d == NDT - 1))
                                    for k in range(2):
                                        eo = ep * 2 + k
                                        ysb = ypool.tile([128, 512], F32,
                                                         tag="ysb", name="ysb")
                                        nc.vector.tensor_copy(ysb[:], yps[k][:])
                                        nc.gpsimd.dma_start(
                                            out[qp * 256 + p * 128:
                                                qp * 256 + (p + 1) * 128,
                                                eo * 512:(eo + 1) * 512], ysb[:])

    nc.compile()
    return nc


def _prep_in_maps(x, cos, sin, attn_mask, wq, wk, wv, wo):
    x = np.asarray(x, np.float32)
    cosT = np.ascontiguousarray(np.asarray(cos[0], np.float32).T)   # [HD, S]
    sinT = np.asarray(sin[0], np.float32).T
    sin_mh = np.ascontiguousarray(
        np.concatenate([-sinT[:64], sinT[64:]], axis=0)).astype(bf16)
    cos_b = cosT.astype(bf16)

    def swz(wt):     # [D, E] -> [128, (d, e)] with row p = wt[d*128+p, e]
        e = wt.shape[1]
        return np.ascontiguousarray(
            wt.reshape(NDT, 128, e).transpose(1, 0, 2).reshape(128, NDT * e))

    wo_tb = swz(np.ascontiguousarray(
        np.asarray(wo, np.float32).T)).astype(bf16)
    wqf = np.asarray(wq, np.float32)
    wkf = np.asarray(wk, np.float32)
    wvf = np.asarray(wv, np.float32)
    masks = [np.ascontiguousarray(
        np.asarray(attn_mask[b], np.float32).reshape(NKT, 128).T)
        for b in range(B)]
    xb = [np.ascontiguousarray(x[b].reshape(TL, D)) for b in range(B)]
    in_maps = []
    for c in range(NCORES):
        b, i = c // GSZ, c % GSZ
        sl = slice(i * EL, (i + 1) * EL)
        in_maps.append({
            "x": xb[b],
            "wq_t": swz(np.ascontiguousarray(wqf[sl].T)).astype(bf16),
            "wk_t": swz(np.ascontiguousarray(wkf[sl].T)).astype(bf16),
            "wv_t": swz(np.ascontiguousarray(wvf[sl].T)).astype(bf16),
            "wo_t": wo_tb,
            "cos_t": cos_b,
            "sin_m": sin_mh,
            "mask_t": masks[b],
        })
    return in_maps


def kernel(x, cos, sin, attn_mask, wq, wk, wv, wo, _trace=False):
    if "nc" not in _CACHE:
        _CACHE["nc"] = _build()
    nc = _CACHE["nc"]
    in_maps = _prep_in_maps(x, cos, sin, attn_mask, wq, wk, wv, wo)
    res = run_bass_kernel_spmd(nc, in_maps, core_ids=list(range(NCORES)),
                               trace=_trace)
    _CACHE["last_result"] = res
    y = np.empty((B, S, D), np.float32)
    for c in range(NCORES):
        oc = np.asarray(res.results[c]["out"], np.float32)
        for qp in range(2):
            for p in range(2):
                t0 = qp * 1024 + c * 128
                y[p, t0:t0 + 128, :] = oc[qp * 256 + p * 128:
                                          qp * 256 + (p + 1) * 128, :]
    return y
